# revision 37
# baseline (speedup 1.0000x reference)
"""Direct Conv2d (NCHW, OIHW, VALID, stride 1) on 8 Trainium2 NeuronCores.

Problem: input [16, 4, 512, 512] f32, filter [8, 4, 3, 3] f32
         -> output [16, 8, 510, 510] f32.

Sharding: data-parallel over batch N: 2 images per core, filter replicated.

Per-core algorithm (all shapes hardcoded):
  Output rows are processed in supertiles of 28 rows = 2 row-interleaved
  sub-blocks: sub-block beta in {0,1} computes rows h0 + 2j + beta for
  j in [0,14). Each sub-block is 3 accumulating float32r matmuls (one per
  filter column shift s, a pure free-dim offset into the shared input
  tile):

    psum_beta[(j,m), w] += sum_{q,c} lhsT[s,beta][(q,c), (j,m)]
                                     * in[c, h0+q, w+s]

  with lhsT[s,beta][(q,c),(j,m)] = filter[m, c, q-2j-beta, s] for
  0 <= q-2j-beta < 3 (banded matrices, built host-side from the
  288-element filter).  K = 30 input rows x 4 channels = 120,
  M = 14 j-rows x 8 out-channels = 112, N = 510 output columns.

  The input tile [120, 512] (partition = q*4+c) is one DMA per supertile
  whose DRAM AP leads with the 30-wide q dim: the HWDGE/SWDGE assign SDMA
  engines by the outer-dim index of the DRAM-side AP, so this spreads
  over all 16 engines.  The two PSUM results are copied (vector engine —
  DMA has no PSUM route) into one SBUF tile [112, 1020] where partition
  (j,m) holds output rows h0+2j and h0+2j+1 back to back, making the
  store's HBM chunks 4080 B contiguous; its DRAM AP leads with the
  14-wide j dim (14 engines).  Input loads go through SWDGE (gpsimd) and
  stores through HWDGE (sync) so descriptor generation is parallel.

  float32r (same bit layout as f32, reduced-precision multiply) streams
  1 PE column/cycle vs 4 for float32; measured output error vs the fp32
  reference is ~1.8e-4 relative.
"""

import os

os.environ.setdefault("MYCRO_LOCAL_CACHE", "1")

import numpy as np

import concourse.bacc as bacc
import concourse.mybir as mybir
import concourse.tile as tile
from concourse.bass_utils import run_bass_kernel_spmd

N_CORES = 8
IMG_PER_CORE = 2
C_IN, H, W = 4, 512, 512
C_OUT, R, S = 8, 3, 3
HO, WO = 510, 510

JB = 14              # output rows per sub-block
QB = JB + R - 1      # 16 input rows per sub-block
KDIM = C_IN * QB     # 64  (matmul contraction dim)
MDIM = C_OUT * JB    # 112 (matmul output partition dim)
SUPER = 2 * JB       # 28 output rows per supertile (2 sub-blocks)
NSUPER = (HO + SUPER - 1) // SUPER  # 19 (last covers rows 504..509)

# Moving/stationary matmul dtype. float32r (same bits as f32, reduced-
# precision multiply) streams 4x faster on the PE than float32.
DT = mybir.dt.float32r

# Set by test harness: TRACE=True -> capture NTFF profile, LAST_EXEC_NS set.
TRACE = False
TRACE_DIR = None
LAST_EXEC_NS = None
LAST_RESULTS = None

_NC_CACHE = {}


def build_wT(filt: np.ndarray) -> np.ndarray:
    """Banded weight matrices [S, 2, 128, MDIM] from filter [8, 4, 3, 3].

    Sub-block beta computes output rows h0 + 2j + beta (row-interleaved),
    so one SBUF partition (j, m) ends up holding two consecutive output
    rows -> 4 KB contiguous HBM chunks on the store.

    K order is q-major (row = q*C_IN + c, q in [0,32)) and M order is
    j-major (col = j*C_OUT + m) so the DRAM-side DMA access patterns lead
    with wide outer dims — the HWDGE assigns SDMA engines by the outer-dim
    index of the DRAM AP.
    """
    wT = np.zeros((S, 2, 128, MDIM), np.float32)
    for s in range(S):
        for beta in range(2):
            for c in range(C_IN):
                for q in range(2 * JB + R + 1):
                    for m in range(C_OUT):
                        for j in range(JB):
                            r = q - 2 * j - beta
                            if 0 <= r < R:
                                wT[s, beta, q * C_IN + c, j * C_OUT + m] = filt[
                                    m, c, r, s
                                ]
    # [partition, (s, beta, col)] so the whole weight set is one contiguous
    # [128, 672] DMA instead of six fragmented ones.
    return np.ascontiguousarray(wT.transpose(2, 0, 1, 3).reshape(128, S * 2 * MDIM))


def conv_body(tc, y, x, wt_d):
    nc = tc.nc
    with (
        tc.tile_pool(name="wt", bufs=1) as wt_pool,
        tc.tile_pool(name="xt", bufs=6) as x_pool,
        tc.tile_pool(name="yt", bufs=6) as y_pool,
        tc.tile_pool(name="ps", bufs=8, space="PSUM") as ps_pool,
    ):
        # Weights: [128, 6*112], chunk (s*2+beta) at cols [k*112, (k+1)*112).
        wt = wt_pool.tile([128, S * 2 * MDIM], DT)
        nc.sync.dma_start(out=wt[:, :], in_=wt_d[:, :])
        for i in range(IMG_PER_CORE):
            for B in range(NSUPER):
                # Last supertile overlaps the previous one (rows 482..509;
                # rows 482..503 are recomputed with identical values) so
                # every supertile is full-size with no edge handling.
                h_base = B * SUPER if B < NSUPER - 1 else HO - SUPER
                # 30 rows cover all taps (q = 2j+beta+r <= 29); partitions
                # 120..127 stay unused on every supertile.
                nq = 30
                xt = x_pool.tile([128, W], DT)
                # dst partition (q*C_IN+c) <-> src element (q, c, w): the
                # 30-wide q dim outermost spreads over all 16 SDMA engines.
                # Loads ride the ACT HWDGE ring; stores use SP-HWDGE and
                # SWDGE, so all three descriptor generators run in parallel.
                nc.scalar.dma_start(
                    out=xt[0 : nq * C_IN, :],
                    in_=x[i, :, h_base : h_base + nq, :].transpose([1, 0, 2]),
                )
                yt = y_pool.tile([MDIM, 2 * WO], mybir.dt.float32)
                for b in range(2):
                    ps = ps_pool.tile([MDIM, WO], mybir.dt.float32)
                    kq = nq * C_IN  # 120 (weight rows beyond are all zero)
                    for s in range(S):
                        k = s * 2 + b
                        nc.tensor.matmul(
                            ps[:, :],
                            lhsT=wt[0:kq, k * MDIM : (k + 1) * MDIM],
                            rhs=xt[0:kq, s : s + WO],
                            start=(s == 0),
                            stop=(s == S - 1),
                        )
                    # partition (j,m): even rows land in cols [0,510),
                    # odd rows in [510,1020) -> 4080B contiguous HBM chunk.
                    nc.vector.tensor_copy(yt[:, b * WO : (b + 1) * WO], ps[:, :])
                # dst element (j, m, (beta,w)) <-> src partition (j*8+m),
                # free (beta,w). HWDGE reaches engines 64-77 (14-wide j
                # outer dim); alternating with SWDGE (spreads by absolute
                # partition over all 16) pulls engines 78/79 into service.
                eng = nc.sync if B % 2 == 0 else nc.gpsimd
                eng.dma_start(
                    out=y[i, :, h_base : h_base + SUPER, :].rearrange(
                        "m (j b) w -> j m (b w)", b=2
                    ),
                    in_=yt[:, :],
                )


def build_nc(enable_asserts: bool = False):
    nc = bacc.Bacc(
        "TRN2",
        target_bir_lowering=False,
        debug=False,
        enable_asserts=enable_asserts,
        num_devices=N_CORES,
    )
    x = nc.dram_tensor("x", [IMG_PER_CORE, C_IN, H, W], DT, kind="ExternalInput").ap()
    wt_d = nc.dram_tensor("wt", [128, S * 2 * MDIM], DT, kind="ExternalInput").ap()
    y = nc.dram_tensor(
        "y", [IMG_PER_CORE, C_OUT, HO, WO], mybir.dt.float32, kind="ExternalOutput"
    ).ap()
    with tile.TileContext(nc) as tc:
        conv_body(tc, y, x, wt_d)
    nc.compile()
    return nc


def kernel(_input: np.ndarray, _filter: np.ndarray) -> np.ndarray:
    global LAST_EXEC_NS, LAST_RESULTS
    _input = np.ascontiguousarray(np.asarray(_input, dtype=np.float32))
    _filter = np.asarray(_filter, dtype=np.float32)

    key = DT
    if key not in _NC_CACHE:
        _NC_CACHE[key] = build_nc()
    nc = _NC_CACHE[key]

    wT = build_wT(_filter)
    in_maps = [
        {
            "x": np.ascontiguousarray(_input[IMG_PER_CORE * i : IMG_PER_CORE * (i + 1)]),
            "wt": wT,
        }
        for i in range(N_CORES)
    ]
    res = run_bass_kernel_spmd(
        nc, in_maps, list(range(N_CORES)), trace=TRACE, tmpdir=TRACE_DIR
    )
    LAST_EXEC_NS = res.exec_time_ns
    LAST_RESULTS = res
    out = np.concatenate([r["y"] for r in res.results], axis=0)
    return out


# revision 41
# speedup vs baseline: 1.0835x; 1.0835x over previous
"""Direct Conv2d (NCHW, OIHW, VALID, stride 1) on 8 Trainium2 NeuronCores.

Problem: input [16, 4, 512, 512] f32, filter [8, 4, 3, 3] f32
         -> output [16, 8, 510, 510] f32.

Sharding: data-parallel over batch N: 2 images per core, filter replicated.

Per-core algorithm (all shapes hardcoded):
  Output rows are processed in supertiles of 28 rows = 2 row-interleaved
  sub-blocks: sub-block beta in {0,1} computes rows h0 + 2j + beta for
  j in [0,14). Each sub-block is 3 accumulating float32r matmuls (one per
  filter column shift s, a pure free-dim offset into the shared input
  tile):

    psum_beta[(j,m), w] += sum_{q,c} lhsT[s,beta][(q,c), (j,m)]
                                     * in[c, h0+q, w+s]

  with lhsT[s,beta][(q,c),(j,m)] = filter[m, c, q-2j-beta, s] for
  0 <= q-2j-beta < 3 (banded matrices, built host-side from the
  288-element filter).  K = 30 input rows x 4 channels = 120,
  M = 14 j-rows x 8 out-channels = 112, N = 510 output columns.

  The input tile [120, 512] (partition = q*4+c) is one DMA per supertile
  whose DRAM AP leads with the 30-wide q dim: the HWDGE/SWDGE assign SDMA
  engines by the outer-dim index of the DRAM-side AP, so this spreads
  over all 16 engines.  The two PSUM results are copied (vector engine —
  DMA has no PSUM route) into one SBUF tile [112, 1020] where partition
  (j,m) holds output rows h0+2j and h0+2j+1 back to back, making the
  store's HBM chunks 4080 B contiguous; its DRAM AP leads with the
  14-wide j dim (14 engines).  Input loads go through SWDGE (gpsimd) and
  stores through HWDGE (sync) so descriptor generation is parallel.

  float32r (same bit layout as f32, reduced-precision multiply) streams
  1 PE column/cycle vs 4 for float32; measured output error vs the fp32
  reference is ~1.8e-4 relative.
"""

import os

os.environ.setdefault("MYCRO_LOCAL_CACHE", "1")

import numpy as np

import concourse.bacc as bacc
import concourse.mybir as mybir
import concourse.tile as tile
from concourse.bass_utils import run_bass_kernel_spmd

N_CORES = 8
IMG_PER_CORE = 2
C_IN, H, W = 4, 512, 512
C_OUT, R, S = 8, 3, 3
HO, WO = 510, 510

JB = 14              # output rows per sub-block
QB = JB + R - 1      # 16 input rows per sub-block
KDIM = C_IN * QB     # 64  (matmul contraction dim)
MDIM = C_OUT * JB    # 112 (matmul output partition dim)
SUPER = 2 * JB       # 28 output rows per supertile (2 sub-blocks)
NSUPER = (HO + SUPER - 1) // SUPER  # 19 (last covers rows 504..509)

# Moving/stationary matmul dtype. float32r (same bits as f32, reduced-
# precision multiply) streams 4x faster on the PE than float32.
DT = mybir.dt.float32r

# Set by test harness: TRACE=True -> capture NTFF profile, LAST_EXEC_NS set.
TRACE = False
TRACE_DIR = None
LAST_EXEC_NS = None
LAST_RESULTS = None

_NC_CACHE = {}


def build_wT(filt: np.ndarray) -> np.ndarray:
    """Banded weight matrices [S, 2, 128, MDIM] from filter [8, 4, 3, 3].

    Sub-block beta computes output rows h0 + 2j + beta (row-interleaved),
    so one SBUF partition (j, m) ends up holding two consecutive output
    rows -> 4 KB contiguous HBM chunks on the store.

    K order is q-major (row = q*C_IN + c, q in [0,32)) and M order is
    j-major (col = j*C_OUT + m) so the DRAM-side DMA access patterns lead
    with wide outer dims — the HWDGE assigns SDMA engines by the outer-dim
    index of the DRAM AP.
    """
    wT = np.zeros((S, 2, 128, MDIM), np.float32)
    for s in range(S):
        for beta in range(2):
            for c in range(C_IN):
                for q in range(2 * JB + R + 1):
                    for m in range(C_OUT):
                        for j in range(JB):
                            r = q - 2 * j - beta
                            if 0 <= r < R:
                                wT[s, beta, q * C_IN + c, j * C_OUT + m] = filt[
                                    m, c, r, s
                                ]
    # [partition, (s, beta, col)] so the whole weight set is one contiguous
    # [128, 672] DMA instead of six fragmented ones.
    return np.ascontiguousarray(wT.transpose(2, 0, 1, 3).reshape(128, S * 2 * MDIM))


def conv_body(tc, y, x, wt_d):
    nc = tc.nc
    with (
        tc.tile_pool(name="wt", bufs=1) as wt_pool,
        tc.tile_pool(name="xt", bufs=6) as x_pool,
        tc.tile_pool(name="yt", bufs=8) as y_pool,
        tc.tile_pool(name="ps", bufs=8, space="PSUM") as ps_pool,
    ):
        # Weights: [128, 6*112], chunk (s*2+beta) at cols [k*112, (k+1)*112).
        wt = wt_pool.tile([128, S * 2 * MDIM], DT)
        nc.scalar.dma_start(out=wt[:, :], in_=wt_d[:, :])
        for i in range(IMG_PER_CORE):
            for B in range(NSUPER):
                # Last supertile overlaps the previous one (rows 482..509;
                # rows 482..503 are recomputed with identical values) so
                # every supertile is full-size with no edge handling.
                h_base = B * SUPER if B < NSUPER - 1 else HO - SUPER
                # 30 rows cover all taps (q = 2j+beta+r <= 29); partitions
                # 120..127 stay unused on every supertile.
                nq = 30
                xt = x_pool.tile([128, W], DT)
                # dst partition (q*C_IN+c) <-> src element (q, c, w): the
                # 30-wide q dim outermost spreads over all 16 SDMA engines.
                # gpsimd = SWDGE: separate descriptor generator from the
                # SP-HWDGE ring used by the output stores.
                nc.gpsimd.dma_start(
                    out=xt[0 : nq * C_IN, :],
                    in_=x[i, :, h_base : h_base + nq, :].transpose([1, 0, 2]),
                )
                yt = y_pool.tile([MDIM, 2 * WO], mybir.dt.float32)
                for b in range(2):
                    ps = ps_pool.tile([MDIM, WO], mybir.dt.float32)
                    kq = nq * C_IN  # 120 (weight rows beyond are all zero)
                    for s in range(S):
                        k = s * 2 + b
                        nc.tensor.matmul(
                            ps[:, :],
                            lhsT=wt[0:kq, k * MDIM : (k + 1) * MDIM],
                            rhs=xt[0:kq, s : s + WO],
                            start=(s == 0),
                            stop=(s == S - 1),
                        )
                    # partition (j,m): even rows land in cols [0,510),
                    # odd rows in [510,1020) -> 4080B contiguous HBM chunk.
                    nc.vector.tensor_copy(yt[:, b * WO : (b + 1) * WO], ps[:, :])
                # dst element (j, m, (beta,w)) <-> src partition (j*8+m),
                # free (beta,w); 14-wide j outermost -> 14 SDMA engines.
                eng = nc.sync
                eng.dma_start(
                    out=y[i, :, h_base : h_base + SUPER, :].rearrange(
                        "m (j b) w -> j m (b w)", b=2
                    ),
                    in_=yt[:, :],
                )


def build_nc(enable_asserts: bool = False):
    nc = bacc.Bacc(
        "TRN2",
        target_bir_lowering=False,
        debug=False,
        enable_asserts=enable_asserts,
        num_devices=N_CORES,
    )
    x = nc.dram_tensor("x", [IMG_PER_CORE, C_IN, H, W], DT, kind="ExternalInput").ap()
    wt_d = nc.dram_tensor("wt", [128, S * 2 * MDIM], DT, kind="ExternalInput").ap()
    y = nc.dram_tensor(
        "y", [IMG_PER_CORE, C_OUT, HO, WO], mybir.dt.float32, kind="ExternalOutput"
    ).ap()
    with tile.TileContext(nc) as tc:
        conv_body(tc, y, x, wt_d)
    nc.compile()
    return nc


def kernel(_input: np.ndarray, _filter: np.ndarray) -> np.ndarray:
    global LAST_EXEC_NS, LAST_RESULTS
    _input = np.ascontiguousarray(np.asarray(_input, dtype=np.float32))
    _filter = np.asarray(_filter, dtype=np.float32)

    key = DT
    if key not in _NC_CACHE:
        _NC_CACHE[key] = build_nc()
    nc = _NC_CACHE[key]

    wT = build_wT(_filter)
    in_maps = [
        {
            "x": np.ascontiguousarray(_input[IMG_PER_CORE * i : IMG_PER_CORE * (i + 1)]),
            "wt": wT,
        }
        for i in range(N_CORES)
    ]
    res = run_bass_kernel_spmd(
        nc, in_maps, list(range(N_CORES)), trace=TRACE, tmpdir=TRACE_DIR
    )
    LAST_EXEC_NS = res.exec_time_ns
    LAST_RESULTS = res
    out = np.concatenate([r["y"] for r in res.results], axis=0)
    return out


# revision 43
# speedup vs baseline: 1.1993x; 1.1069x over previous
"""Direct Conv2d (NCHW, OIHW, VALID, stride 1) on 8 Trainium2 NeuronCores.

Problem: input [16, 4, 512, 512] f32, filter [8, 4, 3, 3] f32
         -> output [16, 8, 510, 510] f32.

Sharding: data-parallel over batch N: 2 images per core, filter replicated.

Per-core algorithm (all shapes hardcoded):
  Output rows are processed in supertiles of 28 rows = 2 row-interleaved
  sub-blocks: sub-block beta in {0,1} computes rows h0 + 2j + beta for
  j in [0,14). Each sub-block is 3 accumulating float32r matmuls (one per
  filter column shift s, a pure free-dim offset into the shared input
  tile):

    psum_beta[(j,m), w] += sum_{q,c} lhsT[s,beta][(q,c), (j,m)]
                                     * in[c, h0+q, w+s]

  with lhsT[s,beta][(q,c),(j,m)] = filter[m, c, q-2j-beta, s] for
  0 <= q-2j-beta < 3 (banded matrices, built host-side from the
  288-element filter).  K = 30 input rows x 4 channels = 120,
  M = 14 j-rows x 8 out-channels = 112, N = 510 output columns.

  The input tile [120, 512] (partition = q*4+c) is one DMA per supertile
  whose DRAM AP leads with the 30-wide q dim: the HWDGE/SWDGE assign SDMA
  engines by the outer-dim index of the DRAM-side AP, so this spreads
  over all 16 engines.  The two PSUM results are copied (vector engine —
  DMA has no PSUM route) into one SBUF tile [112, 1020] where partition
  (j,m) holds output rows h0+2j and h0+2j+1 back to back, making the
  store's HBM chunks 4080 B contiguous; its DRAM AP leads with the
  14-wide j dim (14 engines).  Input loads go through SWDGE (gpsimd) and
  stores through HWDGE (sync) so descriptor generation is parallel.

  float32r (same bit layout as f32, reduced-precision multiply) streams
  1 PE column/cycle vs 4 for float32; measured output error vs the fp32
  reference is ~1.8e-4 relative.
"""

import os

os.environ.setdefault("MYCRO_LOCAL_CACHE", "1")

import numpy as np

import concourse.bacc as bacc
import concourse.mybir as mybir
import concourse.tile as tile
from concourse.bass_utils import run_bass_kernel_spmd

N_CORES = 8
IMG_PER_CORE = 2
C_IN, H, W = 4, 512, 512
C_OUT, R, S = 8, 3, 3
HO, WO = 510, 510

JB = 14              # output rows per sub-block
QB = JB + R - 1      # 16 input rows per sub-block
KDIM = C_IN * QB     # 64  (matmul contraction dim)
MDIM = C_OUT * JB    # 112 (matmul output partition dim)
SUPER = 2 * JB       # 28 output rows per supertile (2 sub-blocks)
NSUPER = (HO + SUPER - 1) // SUPER  # 19 (last covers rows 504..509)

# Moving/stationary matmul dtype. float32r (same bits as f32, reduced-
# precision multiply) streams 4x faster on the PE than float32.
DT = mybir.dt.float32r

# Set by test harness: TRACE=True -> capture NTFF profile, LAST_EXEC_NS set.
TRACE = False
TRACE_DIR = None
LAST_EXEC_NS = None
LAST_RESULTS = None

_NC_CACHE = {}


def build_wT(filt: np.ndarray) -> np.ndarray:
    """Banded weight matrices [S, 2, 128, MDIM] from filter [8, 4, 3, 3].

    Sub-block beta computes output rows h0 + 2j + beta (row-interleaved),
    so one SBUF partition (j, m) ends up holding two consecutive output
    rows -> 4 KB contiguous HBM chunks on the store.

    K order is q-major (row = q*C_IN + c, q in [0,32)) and M order is
    j-major (col = j*C_OUT + m) so the DRAM-side DMA access patterns lead
    with wide outer dims — the HWDGE assigns SDMA engines by the outer-dim
    index of the DRAM AP.
    """
    wT = np.zeros((S, 2, 128, MDIM), np.float32)
    for s in range(S):
        for beta in range(2):
            for c in range(C_IN):
                for q in range(2 * JB + R + 1):
                    for m in range(C_OUT):
                        for j in range(JB):
                            r = q - 2 * j - beta
                            if 0 <= r < R:
                                wT[s, beta, q * C_IN + c, j * C_OUT + m] = filt[
                                    m, c, r, s
                                ]
    # [partition, (s, beta, col)] so the whole weight set is one contiguous
    # [128, 672] DMA instead of six fragmented ones.
    return np.ascontiguousarray(wT.transpose(2, 0, 1, 3).reshape(128, S * 2 * MDIM))


def conv_body(tc, y, x, wt_d):
    nc = tc.nc
    with (
        tc.tile_pool(name="wt", bufs=1) as wt_pool,
        tc.tile_pool(name="xt", bufs=6) as x_pool,
        tc.tile_pool(name="yt", bufs=8) as y_pool,
        tc.tile_pool(name="ps", bufs=8, space="PSUM") as ps_pool,
    ):
        # Weights: [128, 6*112], chunk (s*2+beta) at cols [k*112, (k+1)*112).
        wt = wt_pool.tile([128, S * 2 * MDIM], DT)
        nc.scalar.dma_start(out=wt[:, :], in_=wt_d[:, :])
        for i in range(IMG_PER_CORE):
            for B in range(NSUPER):
                # Supertiles 0..17 cover rows 0..503; the last is a small
                # 6-row tile (jb=3 row-pairs). Its banded lhsT is just a
                # column/row slice of the full weight chunks.
                h_base = B * SUPER
                jb = JB if B < NSUPER - 1 else (HO - (NSUPER - 1) * SUPER) // 2
                # jb row-pairs need input rows q = 2j+beta+r <= 2*jb+1.
                nq = 2 * jb + 2
                xt = x_pool.tile([128, W], DT)
                # dst partition (q*C_IN+c) <-> src element (q, c, w): the
                # 30-wide q dim outermost spreads over all 16 SDMA engines.
                # gpsimd = SWDGE: separate descriptor generator from the
                # SP-HWDGE ring used by the output stores.
                nc.gpsimd.dma_start(
                    out=xt[0 : nq * C_IN, :],
                    in_=x[i, :, h_base : h_base + nq, :].transpose([1, 0, 2]),
                )
                md = jb * C_OUT  # 112, or 24 on the small last tile
                yt = y_pool.tile([MDIM, 2 * WO], mybir.dt.float32)
                for b in range(2):
                    ps = ps_pool.tile([MDIM, WO], mybir.dt.float32)
                    kq = nq * C_IN  # weight rows beyond nq are all zero
                    for s in range(S):
                        k = s * 2 + b
                        nc.tensor.matmul(
                            ps[0:md, :],
                            lhsT=wt[0:kq, k * MDIM : k * MDIM + md],
                            rhs=xt[0:kq, s : s + WO],
                            start=(s == 0),
                            stop=(s == S - 1),
                        )
                    # partition (j,m): even rows land in cols [0,510),
                    # odd rows in [510,1020) -> 4080B contiguous HBM chunk.
                    nc.vector.tensor_copy(yt[0:md, b * WO : (b + 1) * WO], ps[0:md, :])
                # dst element (j, m, (beta,w)) <-> src partition (j*8+m),
                # free (beta,w); jb-wide j outermost -> jb SDMA engines.
                eng = nc.sync
                eng.dma_start(
                    out=y[i, :, h_base : h_base + 2 * jb, :].rearrange(
                        "m (j b) w -> j m (b w)", b=2
                    ),
                    in_=yt[0:md, :],
                )


def build_nc(enable_asserts: bool = False):
    nc = bacc.Bacc(
        "TRN2",
        target_bir_lowering=False,
        debug=False,
        enable_asserts=enable_asserts,
        num_devices=N_CORES,
    )
    x = nc.dram_tensor("x", [IMG_PER_CORE, C_IN, H, W], DT, kind="ExternalInput").ap()
    wt_d = nc.dram_tensor("wt", [128, S * 2 * MDIM], DT, kind="ExternalInput").ap()
    y = nc.dram_tensor(
        "y", [IMG_PER_CORE, C_OUT, HO, WO], mybir.dt.float32, kind="ExternalOutput"
    ).ap()
    with tile.TileContext(nc) as tc:
        conv_body(tc, y, x, wt_d)
    nc.compile()
    return nc


def kernel(_input: np.ndarray, _filter: np.ndarray) -> np.ndarray:
    global LAST_EXEC_NS, LAST_RESULTS
    _input = np.ascontiguousarray(np.asarray(_input, dtype=np.float32))
    _filter = np.asarray(_filter, dtype=np.float32)

    key = DT
    if key not in _NC_CACHE:
        _NC_CACHE[key] = build_nc()
    nc = _NC_CACHE[key]

    wT = build_wT(_filter)
    in_maps = [
        {
            "x": np.ascontiguousarray(_input[IMG_PER_CORE * i : IMG_PER_CORE * (i + 1)]),
            "wt": wT,
        }
        for i in range(N_CORES)
    ]
    res = run_bass_kernel_spmd(
        nc, in_maps, list(range(N_CORES)), trace=TRACE, tmpdir=TRACE_DIR
    )
    LAST_EXEC_NS = res.exec_time_ns
    LAST_RESULTS = res
    out = np.concatenate([r["y"] for r in res.results], axis=0)
    return out
